# revision 10
# baseline (speedup 1.0000x reference)
"""Trainium2 Bass kernel for nn_MessagePassing_7937099563205 (GNN message passing).

Computes out[n, k] = sum_{e : src[e] == n} edge_attrs.flat[k*E + e]
(i.e. jax.ops.segment_sum of edge_attrs.reshape(-1).reshape(F, E).T over
attr_idx[0]) for E=4M edges, F=16 features, N=100000 nodes, on 8 NeuronCores.

Strategy (nodes sharded across cores -> no all-reduce):
  Host:   counting-sort edge ids by destination node; pad each node's edge
          list to a multiple of G=8 ("groups"); schedule groups into rounds
          (round r = every node's r-th group, so indices within a round are
          unique); lay the gathered values out per core as (128 partitions,
          S groups/partition, 16 feats, 8 edges) in round-major token order
          so the device only ever does dense contiguous loads.
  Device: stream the value array (contiguous DMA), tensor_reduce the
          innermost 8-edge axis on the Vector engine (one 16-float partial
          per group), then one dma_scatter_add per round (GPSIMD custom
          DMA, f32 CCE adds into an HBM table). Rounds rotate over 4
          separate tables: concurrent rounds never collide on an address
          (scatter-add RMW is not atomic across DMA engines), and Tile's
          WAW tracking serializes reuse of the same table. A final vector
          add combines the 4 tables into the output.
  Host:   concatenate the 8 per-core node slices, trim to N=100000.
"""

import os
import sys
import numpy as np

_REPO = "/opt/trn_rl_repo"
if _REPO not in sys.path:
    sys.path.insert(0, _REPO)

# ---------------------------------------------------------------- config ----

E = 4_000_000
F = 16
N = 100_000
NC = 8                      # cores
G = 8                       # edges per group (innermost dense-reduce axis)
NPC = 12_512                # nodes per core (8*12512 = 100096 >= N)
NPAD = NC * NPC             # padded node count
TAB_ROWS = 12_544           # per-core table rows (= 98*128), >= NPC, incl trash
TRASH = TAB_ROWS - 1        # dummy node row for padding tokens
NTAB = 4                    # rotating scatter-add tables
CH = 50                     # groups-per-partition processed per DMA chunk

_PROGRAM_CACHE: dict = {}


# ------------------------------------------------------------ the program ---

def build_program(S_rounds, npc=NPC, tab_rows=TAB_ROWS, f=F, g=G, ch=CH,
                  scatter=True, max_call_slots=63):
    # max_call_slots=63 keeps each dma_scatter_add at <=8064 tokens: the TX
    # descriptor stream (2 per token, /16 engines, +1) must fit the 1024-slot
    # SWDGE ring; 8192+ tokens overflow it and kill the NEFF.
    """Build the (SPMD-identical) Bass program for one core."""
    import concourse.bacc as bacc
    import concourse.mybir as mybir
    from concourse import tile

    S = sum(S_rounds)
    T = S * 128
    nc = bacc.Bacc(None, num_swdge_queues=NTAB)
    vals = nc.declare_dram_parameter("vals", [128, S * f * g], mybir.dt.float32,
                                     isOutput=False)
    idxs = nc.declare_dram_parameter("idxs", [128, T // 16], mybir.dt.int16,
                                     isOutput=False)
    out = nc.declare_dram_parameter("out", [tab_rows, f], mybir.dt.float32,
                                    isOutput=True)
    tables = [nc.dram_tensor(f"table{i}", [tab_rows, 64], mybir.dt.float32)
              for i in range(NTAB)]
    srows = tab_rows // 128

    with tile.TileContext(nc) as tc:
        with tc.tile_pool(name="misc", bufs=1) as misc, \
             tc.tile_pool(name="chunks", bufs=4) as chunks:
            # zero the tables' 16-column payload regions
            zt = misc.tile([128, srows * f], mybir.dt.float32)
            nc.vector.memset(zt[:], 0.0)
            for i in range(NTAB):
                nc.sync.dma_start(
                    tables[i][:, 0:f].rearrange("(s p) k -> p s k", p=128),
                    zt[:])

            it = misc.tile([128, T // 16], mybir.dt.int16)
            nc.sync.dma_start(it[:], idxs[:])

            off_tok = 0
            for r, Sr in enumerate(S_rounds):
                red = misc.tile([128, Sr * f], mybir.dt.float32, tag=f"red{r}")
                for off in range(0, Sr, ch):
                    cur = min(ch, Sr - off)
                    t = chunks.tile([128, ch * f * g], mybir.dt.float32,
                                    tag="chunk")
                    o = (off_tok // 128 + off) * f * g
                    nc.sync.dma_start(t[:, :cur * f * g],
                                      vals[:, o:o + cur * f * g])
                    nc.vector.tensor_reduce(
                        red[:, off * f:(off + cur) * f].rearrange(
                            "p (s k) -> p s k", k=f),
                        t[:, :cur * f * g].rearrange(
                            "p (s k m) -> p s k m", k=f, m=g),
                        axis=mybir.AxisListType.X,
                        op=mybir.AluOpType.add,
                    )
                mcs = max_call_slots or Sr
                for soff in range(0, Sr, mcs):
                    scur = min(mcs, Sr - soff)
                    if not scatter:
                        continue
                    nc.gpsimd.dma_scatter_add(
                        tables[r % NTAB][:, 0:f],
                        red[:, soff * f:(soff + scur) * f].rearrange(
                            "p (s k) -> p s k", k=f),
                        it[:, off_tok // 16 + soff * 8:
                           off_tok // 16 + (soff + scur) * 8],
                        scur * 128,   # num_idxs
                        scur * 128,   # num_idxs_reg (pads hit TRASH row)
                        f,            # elem_size
                        elem_step=64,
                        queue_num=r % NTAB,
                    )
                off_tok += Sr * 128

            # combine the 4 tables -> out
            acc = misc.tile([128, srows * f], mybir.dt.float32)
            tb = [misc.tile([128, srows * f], mybir.dt.float32,
                            name=f"tb{i}", tag=f"tb{i}")
                  for i in range(NTAB)]
            for i in range(NTAB):
                nc.sync.dma_start(
                    tb[i][:],
                    tables[i][:, 0:f].rearrange("(s p) k -> p s k", p=128))
            nc.vector.tensor_add(acc[:], tb[0][:], tb[1][:])
            nc.vector.tensor_add(acc[:], acc[:], tb[2][:])
            nc.vector.tensor_add(acc[:], acc[:], tb[3][:])
            nc.sync.dma_start(
                out[:].rearrange("(s p) k -> p s k", p=128), acc[:])

    nc.finalize()
    return nc


def get_program(S_rounds):
    key = ("prog", tuple(S_rounds))
    if key not in _PROGRAM_CACHE:
        _PROGRAM_CACHE[key] = build_program(S_rounds=tuple(S_rounds))
    return _PROGRAM_CACHE[key]


# ------------------------------------------------------- host preprocessing --

def preprocess(edge_attrs, attr_idx, e=E, f=F, g=G, n_cores=NC, npc=NPC,
               trash=TRASH):
    """Counting-sort edges by destination node and build per-core inputs.

    Returns (in_maps, S_rounds). in_maps[c] = {"vals": (128, S*f*g) f32,
    "idxs": (128, S*128//16) int16} with tokens in round-major order
    (round r = each node's r-th group of g edges).
    """
    npad = n_cores * npc
    ea = np.asarray(edge_attrs, dtype=np.float32).reshape(e, f)
    src = np.asarray(attr_idx)[0].astype(np.int32)
    EA2 = ea.reshape(f, e)                      # view; EA2[k, e] = flat[k*E+e]

    order = np.argsort(src, kind="stable").astype(np.int32)
    sidx = src[order]
    counts = np.bincount(src, minlength=npad).astype(np.int64)
    gpn = -(-counts // g)                       # groups per node
    slot_start = np.concatenate(([0], np.cumsum(gpn * g)))
    run_start = np.concatenate(([0], np.cumsum(counts)))
    rank = np.arange(e, dtype=np.int64) - run_start[sidx]
    ng_tot = int(slot_start[-1]) // g
    pos = np.full(ng_tot * g, -1, np.int32)
    pos[slot_start[sidx] + rank] = order
    pos = pos.reshape(ng_tot, g)
    gcum = np.concatenate(([0], np.cumsum(gpn)))

    R = int(gpn.max())
    gpn2 = gpn.reshape(n_cores, npc)
    n_rc = np.stack([(gpn2 > r).sum(axis=1) for r in range(R)])  # (R, cores)
    S_rounds = [int(-(-int(n_rc[r].max()) // 128)) for r in range(R)]
    S = sum(S_rounds)
    T = S * 128

    in_maps = []
    for c in range(n_cores):
        gl = gpn2[c]
        base = c * npc
        tok_g = np.full(T, -1, np.int64)        # global group id per token
        tok_nl = np.full(T, trash, np.int32)    # node-local id per token
        off = 0
        for r in range(R):
            nodes = np.nonzero(gl > r)[0]
            k = len(nodes)
            tok_g[off:off + k] = gcum[base + nodes] + r
            tok_nl[off:off + k] = nodes
            off += S_rounds[r] * 128

        posc = np.full((T, g), -1, np.int32)
        m = tok_g >= 0
        posc[m] = pos[tok_g[m]]

        pc = posc.ravel()
        cl = np.where(pc < 0, 0, pc)
        arrk = EA2[:, cl]                       # (f, T*g)
        arrk[:, pc < 0] = 0.0
        # (f, T, g) -> (T, f, g) token-major; token j = s*128 + p lives at
        # vals[p, s*128 + k*8 + m].
        vals = np.ascontiguousarray(
            arrk.reshape(f, T, g).transpose(1, 0, 2)).reshape(
                S, 128, f * g).transpose(1, 0, 2).reshape(128, S * f * g)
        vals = np.ascontiguousarray(vals)

        wrapped = np.ascontiguousarray(tok_nl.reshape(T // 16, 16).T)
        idxs = np.tile(wrapped, (8, 1)).astype(np.int16)
        in_maps.append({"vals": vals, "idxs": idxs})
    return in_maps, S_rounds


# ---------------------------------------------------------------- kernel ----

def kernel(edge_attrs=None, attr_idx=None, n_nodes=None, **_ignored):
    from concourse.bass_utils import run_bass_kernel_spmd

    in_maps, S_rounds = preprocess(edge_attrs, attr_idx)
    nc = get_program(S_rounds)
    res = run_bass_kernel_spmd(nc, in_maps, core_ids=list(range(NC)))
    out = np.concatenate([res.results[c]["out"][:NPC] for c in range(NC)],
                         axis=0)
    return np.ascontiguousarray(out[:N])
